# revision 1
# baseline (speedup 1.0000x reference)
"""Bilinear 2x upsample (16,3,512,512)->(16,3,1024,1024) on 8 trn2 NeuronCores.

Exact 2x bilinear: src = dst * 0.5, so
  out[2r, 2c]     = x[r, c]
  out[2r, 2c+1]   = 0.5*x[r, c]   + 0.5*x[r, c+1]   (clamped at c=511)
  out[2r+1, *]    = 0.5*row(2r,*) + 0.5*row(2r+2,*) (clamped at r=511)
All scale factors are powers of two, so the kernel reproduces the
reference bit-exactly (mul by 0.5 is exact; the adds round identically).

Sharding: pure data parallel, 2 images (= 6 512x512 planes) per core.

Per-core layout: each plane is loaded as T[128, 4, 512] with input row
r = 4p + b (partition p, free-dim block b). Horizontal interpolation
produces interleaved rows H[128, 4, 1024]; vertical averaging between
consecutive rows happens inside a partition (free-dim block shift) for
3/4 of the rows, and via a one-partition-shift SBUF->SBUF DMA of the
halved rows for the remaining block boundary (row 4p+3 pairs with row
4(p+1) which lives one partition down).
"""

import sys

if "/opt/trn_rl_repo" not in sys.path:
    sys.path.insert(0, "/opt/trn_rl_repo")

import numpy as np

N_CORES = 8
N, C, HI, WI = 16, 3, 512, 512
HO, WO = 1024, 1024
PLANES = (N // N_CORES) * C  # 6 planes per core
P = 128
B = HI // P  # 4 row-blocks per partition

_cached = {}


def _split_excess_waits(nc, max_waits=1):
    """Hoist excess sem waits into no-ops so each instruction carries <=max_waits.

    The walrus build in this container rejects instructions carrying more
    sync-wait commands than the ISA encoding slot count ("Too many sync wait
    commands", e.g. TPB_CTRL holds 1). Tile's scheduler attaches one wait per
    producer proc to a single instruction through an unchecked path. Waiting on
    a chain of same-engine no-ops immediately before the instruction is
    semantically identical (the engine stream is sequential), so move the
    excess waits there.
    """
    import concourse.mybir as mybir

    for f in nc.m.functions:
        for bb in f.blocks:
            insts = bb.instructions
            if not any(
                i.sync_info is not None and len(i.sync_info.on_wait) > max_waits
                for i in insts
            ):
                continue
            new = []
            for inst in insts:
                si = inst.sync_info
                if si is not None and len(si.on_wait) > max_waits:
                    waits = list(si.on_wait)
                    for w in waits[max_waits:]:
                        nop = mybir.InstNoOp(
                            name=nc.get_next_instruction_name(),
                            engine=inst.engine,
                            sync_info=mybir.SyncInfo(on_wait=[w], on_update=[]),
                            bass_nofuse=True,
                        )
                        nc.register_instruction(nop, overwrite=True)
                        new.append(nop)
                    inst.sync_info = mybir.SyncInfo(
                        on_wait=waits[:max_waits], on_update=list(si.on_update)
                    )
                new.append(inst)
            bb.instructions = new


def _build_module(reps=1, bufs=3, store_mode="interleave"):
    import concourse.bass as bass
    import concourse.mybir as mybir
    import concourse.tile as tile

    f32 = mybir.dt.float32
    nc = bass.Bass()
    # x is the host-pre-gathered tile layout: [plane, partition, 5*512]
    # with x[pl, p, b*512 + w] = image[pl, min(4p+b, 511), w].
    x = nc.dram_tensor("x", [PLANES, P, (B + 1) * WI], f32, kind="ExternalInput")
    if store_mode == "interleave":
        out = nc.dram_tensor("out", [PLANES, HO, WO], f32, kind="ExternalOutput")
    else:  # split: separate even/odd row planes, interleaved on the host
        outh = nc.dram_tensor("outh", [PLANES, HI, WO], f32, kind="ExternalOutput")
        outv = nc.dram_tensor("outv", [PLANES, HI, WO], f32, kind="ExternalOutput")

    B5 = B + 1  # 4 owned row-blocks + 1 overlap block (row 4p+4)
    with tile.TileContext(nc) as tc:
        with tc.tile_pool(name="pool", bufs=bufs) as pool:
            for pl in [p for _ in range(reps) for p in range(PLANES)]:
                # ---- load t5[p, b, w] = x[pl, min(4p+b, 511), w], b=0..4.
                # The overlapped+clamped row layout is pre-gathered on the
                # host (see _prep), so this is a single [128, 10KB-contiguous]
                # DMA — the only load shape that runs at full HBM bandwidth
                # here (127-partition, strided, or single-row DMAs measured
                # 10-40x slower).
                t5 = pool.tile([P, B5, WI], f32)
                nc.sync.dma_start(t5[:], x[:][pl].rearrange("p (b w) -> p b w", b=B5))

                # ---- horizontal interpolation -> h5[p, b, 2w interleaved]
                # t5 is halved in place; even output cols restore 2*(x/2) == x.
                nc.scalar.mul(t5[:], t5[:], 0.5)

                h5 = pool.tile([P, B5, WO], f32)
                nc.scalar.mul(h5[:, :, 0:WO:2], t5[:], 2.0)
                nc.vector.tensor_add(
                    h5[:, :, 1 : WO - 1 : 2],
                    t5[:, :, 0 : WI - 1],
                    t5[:, :, 1:WI],
                )
                nc.scalar.mul(h5[:, :, WO - 1 : WO], t5[:, :, WI - 1 : WI], 2.0)

                # ---- vertical averaging: odd out row 2(4p+b)+1 =
                # 0.5*(h(4p+b) + h(4p+b+1)); block shift is within-partition.
                vs = pool.tile([P, B, WO], f32)
                nc.vector.tensor_add(vs[:], h5[:, 0:B, :], h5[:, 1:B5, :])
                nc.scalar.mul(vs[:], vs[:], 0.5)

                # ---- stores
                if store_mode == "interleave":
                    # out row = 8p + 2b + e
                    dst = out[:][pl].rearrange("(p b e) w -> e p b w", b=B, e=2)
                    nc.sync.dma_start(dst[0], h5[:, 0:B, :])
                    nc.sync.dma_start(dst[1], vs[:])
                else:
                    nc.sync.dma_start(
                        outh[:][pl].rearrange("(p b) w -> p b w", b=B),
                        h5[:, 0:B, :],
                    )
                    nc.sync.dma_start(
                        outv[:][pl].rearrange("(p b) w -> p b w", b=B), vs[:]
                    )

    _split_excess_waits(nc)
    nc.finalize()
    return nc


def _get_module():
    if "nc" not in _cached:
        _cached["nc"] = _build_module()
    return _cached["nc"]


_ROW_IDX = np.minimum(
    4 * np.arange(P)[:, None] + np.arange(B + 1)[None, :], HI - 1
)  # [128, 5] source row per (partition, block)


def _prep(planes):
    """[n_planes, 512, 512] image planes -> [n_planes, 128, 2560] tile layout."""
    g = planes[:, _ROW_IDX, :]  # [n, 128, 5, 512]
    return np.ascontiguousarray(g.reshape(planes.shape[0], P, (B + 1) * WI))


def kernel(x, target_height=1024, target_width=1024):
    from concourse.bass_utils import run_bass_kernel_spmd

    assert int(target_height) == HO and int(target_width) == WO
    x = np.asarray(x, dtype=np.float32)
    assert x.shape == (N, C, HI, WI)
    xg = _prep(x.reshape(N * C, HI, WI))  # [48, 128, 2560]

    nc = _get_module()
    per_core = N // N_CORES
    in_maps = [
        {"x": xg[i * PLANES : (i + 1) * PLANES]} for i in range(N_CORES)
    ]
    res = run_bass_kernel_spmd(nc, in_maps, core_ids=list(range(N_CORES)))
    out = np.concatenate(
        [r["out"].reshape(per_core, C, HO, WO) for r in res.results], axis=0
    )
    return out

